# revision 41
# baseline (speedup 1.0000x reference)
"""AttentionPool Trainium2 kernel.

Computes, for x [B, N, D], mask [B, N], q [D]:
    logits = einsum('bnd,d->bn', x, q);  logits[~mask] = -inf
    w = softmax(logits, axis=-1)
    out = einsum('bn,bnd->bd', w, x)

Sharding: data-parallel over B across 8 NeuronCores (4 rows per core).

Position enumeration (per row): n = p*64 + u, with p = SBUF partition
and u = column in [0,64): each partition owns 64 consecutive positions
= one contiguous DRAM run per (partition, row), so every DMA piece is a
single large descriptor per partition.

Design (memory-roofline targeted; trace-measured notes inline):
  - x is masked (zeroed) and cast to fp16 ON THE HOST: the device reads
    16.8 MB/core instead of 33.5 MB f32 (measured ~420 GB/s/core -> the
    x stream is ~40 us). Plain HWDGE (nc.sync) loads - no SWDGE cast
    pass, no gpsimd descriptor-ring hazard. fp16 (not bf16) because
    logit precision from 16-bit inputs is the dominant error term: the
    softmax here is extremely peaked (||q|| ~ 16), so bf16 logits move
    top-2 weight splits by ~2% (rel_err 2.0e-2, at the gate); fp16
    measures 1.9e-3.
  - Masked positions are zeroed in x: their logit becomes exactly 0,
    contributing exp(-shift) to Z (subtracted on the host) and nothing
    to the weighted sum. No device-side mask tensor.
  - The softmax shift is a host-side constant 4.5*||q||, applied as the
    [P, 1] bias of the exp activation (any shift cancels in the host
    division by Z; it only must keep exp() in f32 range).
  - Pass 1 (logits) is SPLIT between DVE and TensorE. The DVE runs a
    custom scan (cumsum of x*q, stride-0 output AP keeps segment ends;
    segment dots = adjacent difference) at its hard cap of 1 elem/cyc
    (HW-probed: custom DVE ops run mode=Disable; the SRC_0_HI crossbar
    lane reads 0.0 outside true 2x mode, so a packed pair-scan is not
    possible). Full pass 1 on DVE = ~70 us > everything else, so the
    LAST GCOLS columns of each row are computed on the TensorE instead:
    the host uploads those columns PRE-TRANSPOSED (xT[b, dlo, c, u, p])
    and the PE does, per (row, c-chunk, u): LDWEIGHTS(xT tile [dlo, p])
    + matmul(rhs=q chunk [dlo, 1]) accumulating the two c-chunks into a
    PSUM logits column [p, 1] - directly in the [p, u] layout that exp
    and pass 2 need. Costs +GCOLS/64 x DMA but removes the same DVE
    fraction; LDW+MM(N=1) is NX-dispatch-bound (~220-320 ns/column).
  - exp on ScalarE: per DVE piece from SBUF, per row for the PE region
    from PSUM (ScalarE sits next to PSUM); both emit w in bf16 (fp16 w
    would flush to zero for plausible shifts; bf16 keeps f32 exponent
    range).
  - DMA orchestration (trace-tuned): each dma_start costs ~0.65 us of
    serial issue time on its engine, so the Sync HWDGE ring carries ONLY
    x pieces (small tensors + xtt + output ride the ACT ring); row-0
    pieces alternate rings to double the early issue rate; xtt loads are
    staggered one-per-row-boundary so at most one competes with the x
    stream for SDMA packets; outputs go on the ACT ring so they don't
    queue behind the x stream (HWDGE rings drain FIFO). Measured exec is
    ~82-89 us (run-to-run HBM/HAM variance ~+-4%): DVE scan stream ~50
    us + ~8 us DVE overhead, ~10 us startup (fixed ~5 us preamble
    barrier + 8-core HBM burst contention on the first pieces), ~7 us
    tail (row-3 drain + PSUM copies + out DMA + end barrier).
  - Pass 2 on TensorE as before: per 2 columns, lhsT = two w columns
    [128, 2] bf16, rhs = their x tiles [128, 512] fp16, one PSUM [2,
    512] accumulation chain per row. Pieces are kept at ~12 cols so PE
    bursts recur every ~3 us and the HAM clock gate stays at 8/8.
  - Z per row via ones^T @ w matmul (out [1, 64] PSUM, host sums the 64
    values): removes the per-piece accum_out + ACTIVATION_READ_
    ACCUMULATOR ops and the end-of-kernel z DMA; the Z columns are
    packed into the same `halves` output DMA.
"""

import numpy as np

B, N, D = 32, 8192, 256
N_CORES = 8
B_LOC = B // N_CORES  # 4
P = 128
T = N // P          # 64 tiles (columns) per row
# trailing columns per row computed on the TensorE; row 0 gets a larger
# share because the DVE would otherwise idle waiting for the 8-core HBM
# burst at stream start
GCOLS_B = (24, 18, 18, 18)
DVE_T_B = tuple(T - g for g in GCOLS_B)
UOFF = tuple(sum(GCOLS_B[:b]) for b in range(4))
UTOT = sum(GCOLS_B)

# per-row DVE piece sizes in COLUMNS (1 col = 128 positions x 256 d).
# Small first pieces start the DVE early; ~12-col steady pieces keep the
# pass-2 matmul bursts ~3 us apart (PE HAM stays warm); the last row
# tapers so the post-stream tail is short.
ROW_PIECES = (
    (2, 4, 6, 10, 10, 8),
    (12, 12, 12, 10),
    (12, 12, 12, 10),
    (12, 12, 10, 6, 4, 2),
)
# PE-region work (lg exp + its pass-2 matmuls + Z part 1) is issued after
# this piece index, pulling it off the end-of-row critical path.
ROW_PIVOT = (4, 2, 2, 2)
ZCOLS = 64  # Z columns packed after the 2*D output halves

_cache = {}

_SCAN_OP_NAME = "ATTNPOOL_MUL_SCAN"


def _register_scan_op():
    """Register a custom DVE op computing scan(add, Src0*Src1) in-process.

    The stock TENSOR_TENSOR_REDUCE / TENSOR_TENSOR_SCAN opcodes crash this
    terminal's ucode; custom-DVE ops ship their own uop tables inside the
    NEFF, so they are self-contained.
    """
    from concourse import dve_ops
    from concourse.dve_spec import AluOp, Spec, Src0, Src1, scan, lower, _has_src1
    from concourse.dve_uop import DveOpSpec

    for op in dve_ops.OPS:
        if op.name == _SCAN_OP_NAME:
            return op
    spec = Spec(
        body=scan(AluOp.ADD, Src0 * Src1),
        reference=lambda in0, in1, c0, c1, c2: np.cumsum(
            in0.astype(np.float32) * in1.astype(np.float32), axis=1, dtype=np.float32
        ),
    )
    row = dve_ops._CUSTOM_DVE_ROW_BASE + len(dve_ops.OPS)
    assert row < 0x20
    shas = {}
    for ver in ("v3", "v4"):
        tmp = DveOpSpec(
            name=_SCAN_OP_NAME,
            opcode=row,
            uops=lower(spec, ver=ver),
            rd1_en=_has_src1(spec),
        )
        shas[ver] = tmp.sha(ver)
    op = dve_ops.DveOp(_SCAN_OP_NAME, spec, subdim=False, uops_sha=shas)
    dve_ops.OPS.append(op)
    dve_ops._SUB_OPCODE_FOR_NAME[_SCAN_OP_NAME] = row
    dve_ops.CUSTOM_DVE_SPECS[_SCAN_OP_NAME] = spec
    return op


def _build():
    import concourse.bass as bass
    import concourse.tile as tile
    from concourse import bacc, mybir, bass_isa

    scan_op = _register_scan_op()

    dt = mybir.dt
    nc = bacc.Bacc(
        "TRN2", target_bir_lowering=False, debug=False, num_devices=N_CORES
    )
    x_d = nc.dram_tensor("x", [B_LOC, N, D], dt.float16, kind="ExternalInput").ap()
    xT_d = nc.dram_tensor(
        "xT", [P, 2, UTOT, P], dt.float16, kind="ExternalInput"
    ).ap()
    nshift_d = nc.dram_tensor(
        "nshift", [P, 1], dt.float32, kind="ExternalInput"
    ).ap()
    q_d = nc.dram_tensor("q", [P, D], dt.float16, kind="ExternalInput").ap()
    qT_d = nc.dram_tensor("qT", [P, 2], dt.float16, kind="ExternalInput").ap()
    ones_d = nc.dram_tensor("ones", [P, 1], dt.float16, kind="ExternalInput").ap()
    out_d = nc.dram_tensor(
        "out", [B_LOC, 2, 2 * D + ZCOLS], dt.float32, kind="ExternalOutput"
    ).ap()

    GE = max(
        DVE_T_B[b] + len(ROW_PIECES[b]) for b in range(B_LOC)
    )  # ends cols: zero col per piece

    with tile.TileContext(nc) as tc:
        with (
            tc.tile_pool(name="singles", bufs=1) as singles,
            tc.tile_pool(name="xrow", bufs=1) as xrow_pool,
            tc.tile_pool(name="xtt", bufs=1) as xtt_pool,
            tc.tile_pool(name="small", bufs=4) as small,
            tc.tile_pool(name="psum", bufs=2, space="PSUM") as psum,
            tc.tile_pool(name="psum1", bufs=2, space="PSUM") as psum1,
        ):
            # row-0 first pieces: the Sync ring carries ONLY x pieces (each
            # issue costs ~0.65 us serially on the issuing engine, so small
            # tensors would delay the stream head by several us).
            xrow0 = x_d[0].rearrange("(p u) d -> p u d", p=P)
            rt0 = xrow_pool.tile([P, T, D], dt.float16, name="rt0")
            k0 = ROW_PIECES[0][0]
            nc.sync.dma_start(rt0[:, 0:k0], xrow0[:, 0:k0])

            # small tensors ride the ACT ring
            qb = singles.tile([P, D], dt.float16)
            nc.scalar.dma_start(qb[:], q_d[:])
            nst = singles.tile([P, 1], dt.float32)
            nc.scalar.dma_start(nst[:], nshift_d[:])
            qT = singles.tile([P, 2], dt.float16)
            nc.scalar.dma_start(qT[:], qT_d[:])
            ones = singles.tile([P, 1], dt.float16)
            nc.scalar.dma_start(ones[:], ones_d[:])

            # persistent per-row ends tiles: zero columns written once; the
            # scans only write the segment-end columns (stride-0 output AP).
            ends_row = [
                singles.tile([P, GE], dt.float32, name=f"ends{j}")
                for j in range(B_LOC)
            ]
            for e in ends_row:
                # GpSimd is otherwise idle; keeps the zero-col writes off
                # the DVE queue entirely
                nc.gpsimd.memset(e[:], 0.0)

            # transposed copies for pass 1 on the PE: ACT HWDGE ring, so
            # they do not delay the x stream on the Sync ring. Only xtt[0]
            # is issued upfront; xtt[b] is issued from inside row b-1's
            # compute section (one 1.3 MB transfer in flight at a time, so
            # the early x pieces are not starved of SDMA packets).
            xtts = [
                xtt_pool.tile([P, 2, GCOLS_B[b], P], dt.float16, name=f"xtt{b}")
                for b in range(B_LOC)
            ]

            rts = [rt0]
            for b in range(B_LOC):
                xrow = x_d[b].rearrange("(p u) d -> p u d", p=P)
                if b > 0:
                    rt = xrow_pool.tile([P, T, D], dt.float16, name=f"rt{b}")
                    rts.append(rt)
                rt = rts[b]
                pieces = ROW_PIECES[b]
                piv = ROW_PIVOT[b]
                start_idx = 1 if b == 0 else 0  # row-0 piece 0 issued above
                off = sum(pieces[:start_idx])
                for pi, k in enumerate(pieces[start_idx:], start=start_idx):
                    # row 0: alternate rings so the issue rate (~0.65 us per
                    # dma_start, serial per engine) doesn't pace the stream
                    eng = nc.scalar if (b == 0 and pi % 2 == 0) else nc.sync
                    eng.dma_start(rt[:, off : off + k], xrow[:, off : off + k])
                    off += k
                    if pi == piv:
                        # PE-region normal-layout columns: needed for the
                        # pass-2 matmuls issued at the pivot
                        nc.sync.dma_start(
                            rt[:, DVE_T_B[b] : T], xrow[:, DVE_T_B[b] : T]
                        )
                if b == 1:
                    # xtt0 after row-1's pieces: by now the stream head has
                    # cleared; needed at row-0's first lg matmuls (~12 us)
                    nc.scalar.dma_start(
                        xtts[0][:], xT_d[:, :, UOFF[0] : UOFF[0] + GCOLS_B[0]]
                    )

            for b in range(B_LOC):
                pieces = ROW_PIECES[b]
                piv = ROW_PIVOT[b]
                g_b, dvt = GCOLS_B[b], DVE_T_B[b]
                assert sum(pieces) == dvt
                rt, xtt, ends = rts[b], xtts[b], ends_row[b]
                logits = small.tile([P, dvt], dt.float32, name="logits")
                w = small.tile([P, T], dt.bfloat16)
                acc = psum.tile([2, 2 * D], dt.float32)
                # one PSUM bank: logits columns [P, g_b] + Z row [1, ZCOLS]
                lgz = psum1.tile([P, max(GCOLS_B) + ZCOLS], dt.float32, name="lgz")
                lg = lgz[:, 0:g_b]
                zp = lgz[0:1, g_b : g_b + ZCOLS]

                # interleave plan: after each piece's pass-2 burst, issue a
                # slice of the row's pass-1 PE columns (data-ready early) so
                # the PE fills the DVE-paced gaps and the HAM stays warm.
                # All g_b columns are issued by the pivot piece.
                g_per = [0] * len(pieces)
                for i in range(g_b):
                    g_per[i % (piv + 1)] += 1

                col0 = 0
                ecol = 0
                gj = 0
                last_col = dvt - 2  # last acc matmul in issue order
                for pi, k in enumerate(pieces):
                    o3 = (
                        ends[:, ecol + 1 : ecol + 1 + k]
                        .rearrange("p (k u) -> p k u", u=1)
                        .broadcast_to([P, k, D])
                    )
                    nc.vector._custom_dve(
                        scan_op,
                        out=o3,
                        in0=rt[:, col0 : col0 + k],
                        in1=qb.rearrange("p (u d) -> p u d", u=1).broadcast_to(
                            [P, k, D]
                        ),
                    )
                    # segment dots = adjacent difference of scan ends; on
                    # GpSimd (otherwise idle) to keep the DVE queue free of
                    # the subtract ops + their semaphore traffic. GpSimd TT
                    # has ~1.7 us Q7 dispatch latency (pipelined mid-row),
                    # so the final pieces of the last row stay on the DVE
                    # where the scan->subtract->exp tail chain is short.
                    nc.gpsimd.tensor_tensor(
                        logits[:, col0 : col0 + k],
                        ends[:, ecol + 1 : ecol + 1 + k],
                        ends[:, ecol : ecol + k],
                        op=mybir.AluOpType.subtract,
                    )
                    nc.scalar.activation(
                        w[:, col0 : col0 + k],
                        logits[:, col0 : col0 + k],
                        mybir.ActivationFunctionType.Exp,
                        bias=nst[:],
                    )
                    for col in range(col0, col0 + k, 2):
                        nc.tensor.matmul(
                            acc[:],
                            w[:, col : col + 2],
                            rt[:, col : col + 2].rearrange("p s d -> p (s d)"),
                            start=(col == 0),
                            stop=(col == last_col),
                        )
                    # pass-1 PE columns for this row, a slice per piece
                    for j in range(gj, gj + g_per[pi]):
                        nc.tensor.matmul(
                            lg[:, j : j + 1],
                            xtt[:, 0, j],
                            qT[:, 0:1],
                            start=True,
                            stop=False,
                        )
                        nc.tensor.matmul(
                            lg[:, j : j + 1],
                            xtt[:, 1, j],
                            qT[:, 1:2],
                            start=False,
                            stop=True,
                        )
                    gj += g_per[pi]
                    col0 += k
                    ecol += k + 1

                    if pi == piv:
                        # pivot: PE-region exp (PSUM -> SBUF), its pass-2
                        # matmuls, and Z part 1 - all off the row tail
                        cpiv = col0
                        if b + 1 < B_LOC:
                            # next row's transposed copy: one xtt transfer
                            # in flight at a time, late enough not to steal
                            # packets from this row's pieces
                            nc.scalar.dma_start(
                                xtts[b + 1][:],
                                xT_d[
                                    :, :, UOFF[b + 1] : UOFF[b + 1] + GCOLS_B[b + 1]
                                ],
                            )
                        nc.scalar.activation(
                            w[:, dvt:T],
                            lg,
                            mybir.ActivationFunctionType.Exp,
                            bias=nst[:],
                        )
                        for col in range(dvt, T, 2):
                            nc.tensor.matmul(
                                acc[:],
                                w[:, col : col + 2],
                                rt[:, col : col + 2].rearrange("p s d -> p (s d)"),
                                start=False,
                                stop=False,
                            )
                        # Z = ones^T @ w, split so only the trailing columns
                        # sit on the row tail (disjoint zp slices)
                        nc.tensor.matmul(
                            zp[:, 0:cpiv],
                            ones[:],
                            w[:, 0:cpiv],
                            start=True,
                            stop=True,
                        )

                # Z part 2: columns not covered at the pivot
                nc.tensor.matmul(
                    zp[:, cpiv:T],
                    ones[:],
                    w[:, cpiv:T],
                    start=True,
                    stop=True,
                )

                halves = small.tile([2, 2 * D + ZCOLS], dt.float32)
                nc.scalar.copy(halves[:, : 2 * D], acc[:])
                nc.scalar.copy(halves[0:1, 2 * D :], zp)
                # ACT HWDGE ring: does not queue behind the x stream (FIFO
                # per physical ring), and issues right after the copies
                nc.scalar.dma_start(out_d[b], halves[:])

    nc.compile()
    return nc


def _prep_core_inputs(x, mask, q):
    """Host-side shard prep. Returns (per-core input dicts, shift)."""
    qb = np.ascontiguousarray(
        np.broadcast_to(q[None, :], (P, D))
    ).astype(np.float16)
    qT = np.ascontiguousarray(q.reshape(2, P).T).astype(np.float16)  # [dlo, c]
    ones = np.ones((P, 1), dtype=np.float16)
    shift = np.float32(4.5 * np.linalg.norm(q.astype(np.float64)))
    nshift = np.full((P, 1), -shift, dtype=np.float32)
    in_maps = []
    for i in range(N_CORES):
        sl = slice(i * B_LOC, (i + 1) * B_LOC)
        # fp16 cast on the host: the device then reads 16.8 MB/core instead
        # of 33.5 MB (the inline SWDGE f32->fp16 cast kept HBM reads f32).
        xm = (x[sl] * mask[sl][:, :, None]).astype(np.float16)
        # pre-transposed trailing columns, per-row widths: [p, u, c, dlo]
        # -> [dlo, c, u, p], rows concatenated along u so each partition
        # (dlo) reads one contiguous run per row slab
        xv = xm.reshape(B_LOC, P, T, 2, P)
        xT = np.ascontiguousarray(
            np.concatenate(
                [
                    xv[b, :, DVE_T_B[b] :].transpose(3, 2, 1, 0)
                    for b in range(B_LOC)
                ],
                axis=2,
            )
        )
        in_maps.append(
            {
                "x": np.ascontiguousarray(xm),
                "xT": xT,
                "nshift": nshift,
                "q": qb,
                "qT": qT,
                "ones": ones,
            }
        )
    return in_maps, shift


def kernel(x, mask, q, _trace=False):
    from concourse.bass_utils import run_bass_kernel_spmd

    x = np.asarray(x, dtype=np.float32)
    mask = np.asarray(mask)
    q = np.asarray(q, dtype=np.float32)
    assert x.shape == (B, N, D) and mask.shape == (B, N) and q.shape == (D,)

    if "nc" not in _cache:
        _cache["nc"] = _build()
    nc = _cache["nc"]

    in_maps, shift = _prep_core_inputs(x, mask, q)
    res = run_bass_kernel_spmd(nc, in_maps, list(range(N_CORES)), trace=_trace)

    # each masked position contributed exp(0 - shift) to Z
    emshift = np.exp(np.float64(-shift))
    n_masked = (~mask).sum(axis=1).astype(np.float64)  # [B]

    out = np.empty((B, D), dtype=np.float32)
    for i in range(N_CORES):
        sl = slice(i * B_LOC, (i + 1) * B_LOC)
        h = res.results[i]["out"]  # [B_LOC, 2, 512+ZCOLS]
        o = h[:, 0, 0:D] + h[:, 1, D : 2 * D]
        z = h[:, 0, 2 * D :].astype(np.float64).sum(axis=1)
        z -= n_masked[sl] * emshift
        out[sl] = o / z[:, None]
    if _trace:
        return out, res
    return out


# revision 43
# speedup vs baseline: 1.0952x; 1.0952x over previous
"""AttentionPool Trainium2 kernel.

Computes, for x [B, N, D], mask [B, N], q [D]:
    logits = einsum('bnd,d->bn', x, q);  logits[~mask] = -inf
    w = softmax(logits, axis=-1)
    out = einsum('bn,bnd->bd', w, x)

Sharding: data-parallel over B across 8 NeuronCores (4 rows per core).

Position enumeration (per row): n = p*64 + u, with p = SBUF partition
and u = column in [0,64): each partition owns 64 consecutive positions
= one contiguous DRAM run per (partition, row), so every DMA piece is a
single large descriptor per partition.

Design (memory-roofline targeted; trace-measured notes inline):
  - x is masked (zeroed) and cast to fp16 ON THE HOST: the device reads
    16.8 MB/core instead of 33.5 MB f32 (measured ~420 GB/s/core -> the
    x stream is ~40 us). Plain HWDGE (nc.sync) loads - no SWDGE cast
    pass, no gpsimd descriptor-ring hazard. fp16 (not bf16) because
    logit precision from 16-bit inputs is the dominant error term: the
    softmax here is extremely peaked (||q|| ~ 16), so bf16 logits move
    top-2 weight splits by ~2% (rel_err 2.0e-2, at the gate); fp16
    measures 1.9e-3.
  - Masked positions are zeroed in x: their logit becomes exactly 0,
    contributing exp(-shift) to Z (subtracted on the host) and nothing
    to the weighted sum. No device-side mask tensor.
  - The softmax shift is a host-side constant 4.5*||q||, applied as the
    [P, 1] bias of the exp activation (any shift cancels in the host
    division by Z; it only must keep exp() in f32 range).
  - Pass 1 (logits) is SPLIT between DVE and TensorE. The DVE runs a
    custom scan (cumsum of x*q, stride-0 output AP keeps segment ends;
    segment dots = adjacent difference) at its hard cap of 1 elem/cyc
    (HW-probed: custom DVE ops run mode=Disable; the SRC_0_HI crossbar
    lane reads 0.0 outside true 2x mode, so a packed pair-scan is not
    possible). Full pass 1 on DVE = ~70 us > everything else, so the
    LAST GCOLS columns of each row are computed on the TensorE instead:
    the host uploads those columns PRE-TRANSPOSED (xT[b, dlo, c, u, p])
    and the PE does, per (row, c-chunk, u): LDWEIGHTS(xT tile [dlo, p])
    + matmul(rhs=q chunk [dlo, 1]) accumulating the two c-chunks into a
    PSUM logits column [p, 1] - directly in the [p, u] layout that exp
    and pass 2 need. Costs +GCOLS/64 x DMA but removes the same DVE
    fraction; LDW+MM(N=1) is NX-dispatch-bound (~220-320 ns/column).
  - exp on ScalarE: per DVE piece from SBUF, per row for the PE region
    from PSUM (ScalarE sits next to PSUM); both emit w in bf16 (fp16 w
    would flush to zero for plausible shifts; bf16 keeps f32 exponent
    range).
  - DMA orchestration (trace-tuned): each dma_start costs ~0.65 us of
    serial issue time on its engine, so the Sync HWDGE ring carries ONLY
    x pieces (small tensors + xtt + output ride the ACT ring); row-0
    pieces alternate rings to double the early issue rate; xtt loads are
    staggered one-per-row-boundary so at most one competes with the x
    stream for SDMA packets; outputs go on the ACT ring so they don't
    queue behind the x stream (HWDGE rings drain FIFO). Measured exec is
    ~82-89 us (run-to-run HBM/HAM variance ~+-4%): DVE scan stream ~50
    us + ~8 us DVE overhead, ~10 us startup (fixed ~5 us preamble
    barrier + 8-core HBM burst contention on the first pieces), ~7 us
    tail (row-3 drain + PSUM copies + out DMA + end barrier).
  - Pass 2 on TensorE as before: per 2 columns, lhsT = two w columns
    [128, 2] bf16, rhs = their x tiles [128, 512] fp16, one PSUM [2,
    512] accumulation chain per row. Pieces are kept at ~12 cols so PE
    bursts recur every ~3 us and the HAM clock gate stays at 8/8.
  - Z per row via ones^T @ w matmul (out [1, 64] PSUM, host sums the 64
    values): removes the per-piece accum_out + ACTIVATION_READ_
    ACCUMULATOR ops and the end-of-kernel z DMA; the Z columns are
    packed into the same `halves` output DMA.
"""

import numpy as np

B, N, D = 32, 8192, 256
N_CORES = 8
B_LOC = B // N_CORES  # 4
P = 128
T = N // P          # 64 tiles (columns) per row
# trailing columns per row computed on the TensorE (asymmetric per-row
# splits measured WORSE: the extra early DMA slows the DVE scans via
# SBUF write-port contention)
GCOLS_B = (18, 18, 18, 18)
DVE_T_B = tuple(T - g for g in GCOLS_B)
UOFF = tuple(sum(GCOLS_B[:b]) for b in range(4))
UTOT = sum(GCOLS_B)

# per-row DVE piece sizes in COLUMNS (1 col = 128 positions x 256 d).
# Small first pieces start the DVE early; ~12-col steady pieces keep the
# pass-2 matmul bursts ~3 us apart (PE HAM stays warm); the last row
# tapers so the post-stream tail is short.
ROW_PIECES = (
    (2, 4, 6, 10, 12, 12),
    (12, 12, 12, 10),
    (12, 12, 12, 10),
    (12, 12, 10, 6, 4, 2),
)
# PE-region work (lg exp + its pass-2 matmuls + Z part 1) is issued after
# this piece index, pulling it off the end-of-row critical path.
ROW_PIVOT = (4, 2, 2, 2)
ZCOLS = 64  # Z columns packed after the 2*D output halves

_cache = {}

_SCAN_OP_NAME = "ATTNPOOL_MUL_SCAN"


def _register_scan_op():
    """Register a custom DVE op computing scan(add, Src0*Src1) in-process.

    The stock TENSOR_TENSOR_REDUCE / TENSOR_TENSOR_SCAN opcodes crash this
    terminal's ucode; custom-DVE ops ship their own uop tables inside the
    NEFF, so they are self-contained.
    """
    from concourse import dve_ops
    from concourse.dve_spec import AluOp, Spec, Src0, Src1, scan, lower, _has_src1
    from concourse.dve_uop import DveOpSpec

    for op in dve_ops.OPS:
        if op.name == _SCAN_OP_NAME:
            return op
    spec = Spec(
        body=scan(AluOp.ADD, Src0 * Src1),
        reference=lambda in0, in1, c0, c1, c2: np.cumsum(
            in0.astype(np.float32) * in1.astype(np.float32), axis=1, dtype=np.float32
        ),
    )
    row = dve_ops._CUSTOM_DVE_ROW_BASE + len(dve_ops.OPS)
    assert row < 0x20
    shas = {}
    for ver in ("v3", "v4"):
        tmp = DveOpSpec(
            name=_SCAN_OP_NAME,
            opcode=row,
            uops=lower(spec, ver=ver),
            rd1_en=_has_src1(spec),
        )
        shas[ver] = tmp.sha(ver)
    op = dve_ops.DveOp(_SCAN_OP_NAME, spec, subdim=False, uops_sha=shas)
    dve_ops.OPS.append(op)
    dve_ops._SUB_OPCODE_FOR_NAME[_SCAN_OP_NAME] = row
    dve_ops.CUSTOM_DVE_SPECS[_SCAN_OP_NAME] = spec
    return op


def _build():
    import concourse.bass as bass
    import concourse.tile as tile
    from concourse import bacc, mybir, bass_isa

    scan_op = _register_scan_op()

    dt = mybir.dt
    nc = bacc.Bacc(
        "TRN2", target_bir_lowering=False, debug=False, num_devices=N_CORES
    )
    x_d = nc.dram_tensor("x", [B_LOC, N, D], dt.float16, kind="ExternalInput").ap()
    xT_d = nc.dram_tensor(
        "xT", [P, 2, UTOT, P], dt.float16, kind="ExternalInput"
    ).ap()
    nshift_d = nc.dram_tensor(
        "nshift", [P, 1], dt.float32, kind="ExternalInput"
    ).ap()
    q_d = nc.dram_tensor("q", [P, D], dt.float16, kind="ExternalInput").ap()
    qT_d = nc.dram_tensor("qT", [P, 2], dt.float16, kind="ExternalInput").ap()
    ones_d = nc.dram_tensor("ones", [P, 1], dt.float16, kind="ExternalInput").ap()
    out_d = nc.dram_tensor(
        "out", [B_LOC, 2, 2 * D + ZCOLS], dt.float32, kind="ExternalOutput"
    ).ap()

    GE = max(
        DVE_T_B[b] + len(ROW_PIECES[b]) for b in range(B_LOC)
    )  # ends cols: zero col per piece

    with tile.TileContext(nc) as tc:
        with (
            tc.tile_pool(name="singles", bufs=1) as singles,
            tc.tile_pool(name="xrow", bufs=1) as xrow_pool,
            tc.tile_pool(name="xtt", bufs=1) as xtt_pool,
            tc.tile_pool(name="small", bufs=4) as small,
            tc.tile_pool(name="psum", bufs=2, space="PSUM") as psum,
            tc.tile_pool(name="psum1", bufs=2, space="PSUM") as psum1,
        ):
            # row-0 first pieces: the Sync ring carries ONLY x pieces (each
            # issue costs ~0.65 us serially on the issuing engine, so small
            # tensors would delay the stream head by several us).
            xrow0 = x_d[0].rearrange("(p u) d -> p u d", p=P)
            rt0 = xrow_pool.tile([P, T, D], dt.float16, name="rt0")
            k0 = ROW_PIECES[0][0]
            nc.sync.dma_start(rt0[:, 0:k0], xrow0[:, 0:k0])

            # small tensors ride the ACT ring
            qb = singles.tile([P, D], dt.float16)
            nc.scalar.dma_start(qb[:], q_d[:])
            nst = singles.tile([P, 1], dt.float32)
            nc.scalar.dma_start(nst[:], nshift_d[:])
            qT = singles.tile([P, 2], dt.float16)
            nc.scalar.dma_start(qT[:], qT_d[:])
            ones = singles.tile([P, 1], dt.float16)
            nc.scalar.dma_start(ones[:], ones_d[:])

            # persistent per-row ends tiles: zero columns written once; the
            # scans only write the segment-end columns (stride-0 output AP).
            ends_row = [
                singles.tile([P, GE], dt.float32, name=f"ends{j}")
                for j in range(B_LOC)
            ]
            for e in ends_row:
                # GpSimd is otherwise idle; keeps the zero-col writes off
                # the DVE queue entirely
                nc.gpsimd.memset(e[:], 0.0)

            # transposed copies for pass 1 on the PE: ACT HWDGE ring, so
            # they do not delay the x stream on the Sync ring. Only xtt[0]
            # is issued upfront; xtt[b] is issued from inside row b-1's
            # compute section (one 1.3 MB transfer in flight at a time, so
            # the early x pieces are not starved of SDMA packets).
            xtts = [
                xtt_pool.tile([P, 2, GCOLS_B[b], P], dt.float16, name=f"xtt{b}")
                for b in range(B_LOC)
            ]

            rts = [rt0]
            for b in range(B_LOC):
                xrow = x_d[b].rearrange("(p u) d -> p u d", p=P)
                if b > 0:
                    rt = xrow_pool.tile([P, T, D], dt.float16, name=f"rt{b}")
                    rts.append(rt)
                rt = rts[b]
                pieces = ROW_PIECES[b]
                piv = ROW_PIVOT[b]
                start_idx = 1 if b == 0 else 0  # row-0 piece 0 issued above
                off = sum(pieces[:start_idx])
                for pi, k in enumerate(pieces[start_idx:], start=start_idx):
                    # row 0: alternate rings so the issue rate (~0.65 us per
                    # dma_start, serial per engine) doesn't pace the stream
                    eng = nc.scalar if (b == 0 and pi % 2 == 0) else nc.sync
                    eng.dma_start(rt[:, off : off + k], xrow[:, off : off + k])
                    off += k
                    if pi == piv:
                        # PE-region normal-layout columns: needed for the
                        # pass-2 matmuls issued at the pivot
                        nc.sync.dma_start(
                            rt[:, DVE_T_B[b] : T], xrow[:, DVE_T_B[b] : T]
                        )
                if b == 1:
                    # xtt0 after row-1's pieces: by now the stream head has
                    # cleared; needed at row-0's first lg matmuls (~12 us)
                    nc.scalar.dma_start(
                        xtts[0][:], xT_d[:, :, UOFF[0] : UOFF[0] + GCOLS_B[0]]
                    )

            for b in range(B_LOC):
                pieces = ROW_PIECES[b]
                piv = ROW_PIVOT[b]
                g_b, dvt = GCOLS_B[b], DVE_T_B[b]
                assert sum(pieces) == dvt
                rt, xtt, ends = rts[b], xtts[b], ends_row[b]
                logits = small.tile([P, dvt], dt.float32, name="logits")
                w = small.tile([P, T], dt.bfloat16)
                acc = psum.tile([2, 2 * D], dt.float32)
                # one PSUM bank: logits columns [P, g_b] + Z row [1, ZCOLS]
                lgz = psum1.tile([P, max(GCOLS_B) + ZCOLS], dt.float32, name="lgz")
                lg = lgz[:, 0:g_b]
                zp = lgz[0:1, g_b : g_b + ZCOLS]

                # interleave plan: after each piece's pass-2 burst, issue a
                # slice of the row's pass-1 PE columns (data-ready early) so
                # the PE fills the DVE-paced gaps and the HAM stays warm.
                # All g_b columns are issued by the pivot piece.
                g_per = [0] * len(pieces)
                for i in range(g_b):
                    g_per[i % (piv + 1)] += 1

                col0 = 0
                ecol = 0
                gj = 0
                last_col = dvt - 2  # last acc matmul in issue order
                for pi, k in enumerate(pieces):
                    o3 = (
                        ends[:, ecol + 1 : ecol + 1 + k]
                        .rearrange("p (k u) -> p k u", u=1)
                        .broadcast_to([P, k, D])
                    )
                    nc.vector._custom_dve(
                        scan_op,
                        out=o3,
                        in0=rt[:, col0 : col0 + k],
                        in1=qb.rearrange("p (u d) -> p u d", u=1).broadcast_to(
                            [P, k, D]
                        ),
                    )
                    # segment dots = adjacent difference of scan ends; on
                    # GpSimd (otherwise idle) to keep the DVE queue free of
                    # the subtract ops + their semaphore traffic. GpSimd TT
                    # has ~1.7 us Q7 dispatch latency (pipelined mid-row),
                    # so the final pieces of the last row stay on the DVE
                    # where the scan->subtract->exp tail chain is short.
                    nc.gpsimd.tensor_tensor(
                        logits[:, col0 : col0 + k],
                        ends[:, ecol + 1 : ecol + 1 + k],
                        ends[:, ecol : ecol + k],
                        op=mybir.AluOpType.subtract,
                    )
                    nc.scalar.activation(
                        w[:, col0 : col0 + k],
                        logits[:, col0 : col0 + k],
                        mybir.ActivationFunctionType.Exp,
                        bias=nst[:],
                    )
                    for col in range(col0, col0 + k, 2):
                        nc.tensor.matmul(
                            acc[:],
                            w[:, col : col + 2],
                            rt[:, col : col + 2].rearrange("p s d -> p (s d)"),
                            start=(col == 0),
                            stop=(col == last_col),
                        )
                    # pass-1 PE columns for this row, a slice per piece
                    for j in range(gj, gj + g_per[pi]):
                        nc.tensor.matmul(
                            lg[:, j : j + 1],
                            xtt[:, 0, j],
                            qT[:, 0:1],
                            start=True,
                            stop=False,
                        )
                        nc.tensor.matmul(
                            lg[:, j : j + 1],
                            xtt[:, 1, j],
                            qT[:, 1:2],
                            start=False,
                            stop=True,
                        )
                    gj += g_per[pi]
                    col0 += k
                    ecol += k + 1

                    if pi == piv:
                        # pivot: PE-region exp (PSUM -> SBUF), its pass-2
                        # matmuls, and Z part 1 - all off the row tail
                        cpiv = col0
                        if b + 1 < B_LOC:
                            # next row's transposed copy: one xtt transfer
                            # in flight at a time, late enough not to steal
                            # packets from this row's pieces
                            nc.scalar.dma_start(
                                xtts[b + 1][:],
                                xT_d[
                                    :, :, UOFF[b + 1] : UOFF[b + 1] + GCOLS_B[b + 1]
                                ],
                            )
                        nc.scalar.activation(
                            w[:, dvt:T],
                            lg,
                            mybir.ActivationFunctionType.Exp,
                            bias=nst[:],
                        )
                        for col in range(dvt, T, 2):
                            nc.tensor.matmul(
                                acc[:],
                                w[:, col : col + 2],
                                rt[:, col : col + 2].rearrange("p s d -> p (s d)"),
                                start=False,
                                stop=False,
                            )
                        # Z = ones^T @ w, split so only the trailing columns
                        # sit on the row tail (disjoint zp slices)
                        nc.tensor.matmul(
                            zp[:, 0:cpiv],
                            ones[:],
                            w[:, 0:cpiv],
                            start=True,
                            stop=True,
                        )

                # Z part 2: columns not covered at the pivot
                nc.tensor.matmul(
                    zp[:, cpiv:T],
                    ones[:],
                    w[:, cpiv:T],
                    start=True,
                    stop=True,
                )

                halves = small.tile([2, 2 * D + ZCOLS], dt.float32)
                nc.scalar.copy(halves[:, : 2 * D], acc[:])
                nc.scalar.copy(halves[0:1, 2 * D :], zp)
                # ACT HWDGE ring: does not queue behind the x stream (FIFO
                # per physical ring), and issues right after the copies
                nc.scalar.dma_start(out_d[b], halves[:])

    nc.compile()
    return nc


def _prep_core_inputs(x, mask, q):
    """Host-side shard prep. Returns (per-core input dicts, shift)."""
    qb = np.ascontiguousarray(
        np.broadcast_to(q[None, :], (P, D))
    ).astype(np.float16)
    qT = np.ascontiguousarray(q.reshape(2, P).T).astype(np.float16)  # [dlo, c]
    ones = np.ones((P, 1), dtype=np.float16)
    shift = np.float32(4.5 * np.linalg.norm(q.astype(np.float64)))
    nshift = np.full((P, 1), -shift, dtype=np.float32)
    in_maps = []
    for i in range(N_CORES):
        sl = slice(i * B_LOC, (i + 1) * B_LOC)
        # fp16 cast on the host: the device then reads 16.8 MB/core instead
        # of 33.5 MB (the inline SWDGE f32->fp16 cast kept HBM reads f32).
        xm = (x[sl] * mask[sl][:, :, None]).astype(np.float16)
        # pre-transposed trailing columns, per-row widths: [p, u, c, dlo]
        # -> [dlo, c, u, p], rows concatenated along u so each partition
        # (dlo) reads one contiguous run per row slab
        xv = xm.reshape(B_LOC, P, T, 2, P)
        xT = np.ascontiguousarray(
            np.concatenate(
                [
                    xv[b, :, DVE_T_B[b] :].transpose(3, 2, 1, 0)
                    for b in range(B_LOC)
                ],
                axis=2,
            )
        )
        in_maps.append(
            {
                "x": np.ascontiguousarray(xm),
                "xT": xT,
                "nshift": nshift,
                "q": qb,
                "qT": qT,
                "ones": ones,
            }
        )
    return in_maps, shift


def kernel(x, mask, q, _trace=False):
    from concourse.bass_utils import run_bass_kernel_spmd

    x = np.asarray(x, dtype=np.float32)
    mask = np.asarray(mask)
    q = np.asarray(q, dtype=np.float32)
    assert x.shape == (B, N, D) and mask.shape == (B, N) and q.shape == (D,)

    if "nc" not in _cache:
        _cache["nc"] = _build()
    nc = _cache["nc"]

    in_maps, shift = _prep_core_inputs(x, mask, q)
    res = run_bass_kernel_spmd(nc, in_maps, list(range(N_CORES)), trace=_trace)

    # each masked position contributed exp(0 - shift) to Z
    emshift = np.exp(np.float64(-shift))
    n_masked = (~mask).sum(axis=1).astype(np.float64)  # [B]

    out = np.empty((B, D), dtype=np.float32)
    for i in range(N_CORES):
        sl = slice(i * B_LOC, (i + 1) * B_LOC)
        h = res.results[i]["out"]  # [B_LOC, 2, 512+ZCOLS]
        o = h[:, 0, 0:D] + h[:, 1, D : 2 * D]
        z = h[:, 0, 2 * D :].astype(np.float64).sum(axis=1)
        z -= n_masked[sl] * emshift
        out[sl] = o / z[:, None]
    if _trace:
        return out, res
    return out
